# revision 17
# baseline (speedup 1.0000x reference)
"""Bass/Trainium2 kernel for nn_Attention_21354577395789.

Reference computation (B=16, S=2048, H=1024, D=2H=2048):
    h      = broadcast(hidden[1, 2H]) -> [B, S, 2H]
    cat    = concat([h, enc], -1)                    [B, S, 4H]
    energy = tanh(cat @ attn_w.T + attn_b)           [B, S, H]
    scores = energy @ v_w.T                          [B, S, 1]
    attn   = softmax(scores, axis=1)
    ctx    = attn^T @ enc                            [B, 1, 2H]

Algebraic simplifications:
  * attn_w = [W_h | W_e] along its 4H input dim; c = hidden @ W_h.T + attn_b
    is a shared [H] vector, computed host-side. The surviving big matmul is
    enc @ W_e.T.
  * That matmul runs in fp8 e4m3 with MatmulPerfMode.DoubleRow (2x PE rate).
    W_e is quantized host-side with v-weighted error-feedback rounding (the
    rounding errors are steered to cancel in the score functional), enc with
    plain RTN at scale 16; the 1/(16*64) descale folds into the tanh's input
    scale on the ACT engine.
  * The v-reduction (scores = energy @ v) runs on the Vector engine as fused
    per-partition multiply-accumulate, followed by a single ones-vector
    matmul per 512-block for the partition sum (instead of 8 skinny PE
    matmuls per block).
  * Softmax + context are computed per S-half with online max-combining, so
    the second half of each batch's context matmuls (+ the whole context of
    the previous batch) overlap the next energy block; only a small combine
    remains in the tail.

Distribution: pure data-parallel over B across 8 NeuronCores (2 batches per
core), no collectives.
"""

import os

import numpy as np
import ml_dtypes

B, S, H = 16, 2048, 1024
D = 2 * H
N_CORES = 8
BPC = B // N_CORES   # batches per core = 2
JT = H // 128        # 8 output j-tiles
PAIRS = D // 256     # 8 k-pair chunks (DoubleRow consumes 256 of D at a time)
HALF = S // 2        # 1024
NT = 512             # moving block (t columns per energy matmul)
SW = 64.0            # fp8 scale for W_e
SE = 16.0            # fp8 scale for enc
ISCALE = 1.0 / (SW * SE)

BF16 = ml_dtypes.bfloat16
F8 = ml_dtypes.float8_e4m3

_cache = {}


def _ef_quant_w(W, wvec, scale):
    """Quantize W*scale to fp8 e4m3, rounding row-by-row so the wvec-weighted
    rounding error per column stays near zero (greedy error feedback)."""
    Xs = (W * scale).astype(np.float32)
    out = np.empty_like(Xs)
    C = np.zeros(Xs.shape[1], dtype=np.float64)
    for i in range(Xs.shape[0]):
        x = Xs[i]
        n = x.astype(F8).astype(np.float32)
        err_n = n - x
        ulp = np.maximum(np.abs(n) * 2.0**-3, 2.0**-9 * scale / 64.0 * 8.0)
        alt = np.where(err_n > 0, x - ulp, x + ulp).astype(F8).astype(np.float32)
        err_a = alt - x
        w = wvec[i]
        pick_alt = np.abs(C + w * err_a) < np.abs(C + w * err_n)
        out[i] = np.where(pick_alt, alt, n)
        C = C + w * np.where(pick_alt, err_a, err_n)
    return out.astype(F8)


def _build():
    import concourse.bacc as bacc
    import concourse.tile as tile
    from concourse import mybir

    nc = bacc.Bacc("TRN2", target_bir_lowering=False, debug=False)
    dt = mybir.dt
    AF = mybir.ActivationFunctionType
    AX = mybir.AxisListType
    ALU = mybir.AluOpType
    DR = mybir.MatmulPerfMode.DoubleRow

    # DRAM params (per core). Layouts are precomputed host-side:
    #   encT8[b, h, p, pair, i, s] = fp8(enc[b, h*1024+s, pair*256+i*128+p]*SE)
    #   encN2[b, t2, p, c, d]      = bf16(enc[b, t2*256+c*128+p, d])
    #   w8[p, pair, i, j]          = ef-fp8(W_e[j, pair*256+i*128+p]*SW)
    encT8 = nc.declare_dram_parameter("encT8", [BPC, 2, 128, PAIRS, 2, HALF], dt.float8e4, isOutput=False)
    encN2 = nc.declare_dram_parameter("encN2", [BPC, 8, 128, 2, D], dt.bfloat16, isOutput=False)
    w8 = nc.declare_dram_parameter("w8", [JT, 128, PAIRS, 2, 128], dt.float8e4, isOutput=False)
    c_cols = nc.declare_dram_parameter("c_cols", [128, JT], dt.float32, isOutput=False)
    v_cols = nc.declare_dram_parameter("v_cols", [128, JT], dt.float32, isOutput=False)
    out = nc.declare_dram_parameter("out", [BPC, D], dt.float32, isOutput=True)

    with tile.TileContext(nc) as tc:
        with (
            tc.tile_pool(name="const", bufs=1) as wpool,
            tc.tile_pool(name="enct", bufs=3) as enct_pool,
            tc.tile_pool(name="encn", bufs=8) as encn_pool,
            tc.tile_pool(name="energy", bufs=4) as epool,
            tc.tile_pool(name="accs", bufs=4) as apool,
            tc.tile_pool(name="rows", bufs=2) as rpool,
            tc.tile_pool(name="wcols", bufs=2) as wcpool,
            tc.tile_pool(name="small", bufs=28) as spool,
            tc.tile_pool(name="psum_e", bufs=4, space="PSUM") as pe_pool,
            tc.tile_pool(name="psum_x", bufs=4, space="PSUM") as xp_pool,
        ):
            # ---- resident constants ----------------------------------
            w8_sb = wpool.tile([128, JT, PAIRS, 2, 128], dt.float8e4, tag="w8")
            c_sb = wpool.tile([128, JT], dt.float32, tag="c")
            v_sb = wpool.tile([128, JT], dt.float32, tag="v")
            ones_sb = wpool.tile([128, 1], dt.bfloat16, tag="ones")
            nc.vector.memset(ones_sb[:], 1.0)
            neg4_sb = wpool.tile([1, 1], dt.float32, tag="neg4")
            nc.vector.memset(neg4_sb[:], -4.0)
            pos4_sb = wpool.tile([1, 1], dt.float32, tag="pos4")
            nc.vector.memset(pos4_sb[:], 4.0)

            halves = [(b, h) for b in range(BPC) for h in range(2)]

            # ---- DMA prefetch helpers --------------------------------
            enct_tiles = {}

            def fetch_enct(b, h, startup=False):
                t = enct_pool.tile(
                    [128, PAIRS, 2, HALF], dt.float8e4, tag="enct",
                    name=f"enct{b}{h}",
                )
                if not startup:
                    for tb in range(2):
                        for pair in range(PAIRS):
                            nc.sync.dma_start(
                                t[:, pair, :, tb * NT : (tb + 1) * NT],
                                encT8.ap()[b, h, :, pair, :, tb * NT : (tb + 1) * NT],
                            )
                    enct_tiles[(b, h)] = t
                    return
                # startup: dual-queue issue, critical pieces first.
                # jj0's chain needs w8[jj0] + all (pair, tb=0) pieces.
                pieces = []
                for half_p in range(2):
                    pieces.append((w8_sb[:, 0, half_p * 4 : half_p * 4 + 4, :, :],
                                   w8.ap()[0, :, half_p * 4 : half_p * 4 + 4, :, :]))
                for pair in range(PAIRS):
                    pieces.append((t[:, pair, :, 0:NT],
                                   encT8.ap()[b, h, :, pair, :, 0:NT]))
                pieces.append((c_sb[:], c_cols.ap()[:]))
                pieces.append((v_sb[:], v_cols.ap()[:]))
                for pair in range(PAIRS):
                    pieces.append((t[:, pair, :, NT : 2 * NT],
                                   encT8.ap()[b, h, :, pair, :, NT : 2 * NT]))
                for jj in range(1, JT):
                    for half_p in range(2):
                        pieces.append((
                            w8_sb[:, jj, half_p * 4 : half_p * 4 + 4, :, :],
                            w8.ap()[jj, :, half_p * 4 : half_p * 4 + 4, :, :],
                        ))
                for k, (dst, srcp) in enumerate(pieces):
                    eng = nc.sync if k % 2 == 0 else nc.scalar
                    eng.dma_start(dst, srcp)
                enct_tiles[(b, h)] = t

            encn_tiles = {}

            def fetch_encn(b, h):
                # 4 slabs of 256 s-rows each cover one half
                for t2 in range(h * 4, h * 4 + 4):
                    t = encn_pool.tile(
                        [128, 2, D], dt.bfloat16, tag="encn",
                        name=f"encn{b}{t2}",
                    )
                    for c in range(2):
                        nc.sync.dma_start(
                            t[:, c, :], encN2.ap()[b, t2, :, c, :]
                        )
                    encn_tiles[(b, t2)] = t

            # ---- softmax units + context jobs ------------------------
            # A "unit" is a softmax segment (full half or one 512-block).
            # Each unit's context (exp-weights @ enc, unnormalized) is either
            # stored to an SBUF row or, for the last unit, combined online
            # with the batch's merged row: out = (f1*row + f2*ctx_fin)/Z.
            class Unit:
                pass

            row_units = {}   # b -> merged row-unit

            class CtxJob:
                def __init__(self, b, unit, final, register_row=True):
                    self.b, self.u, self.final = b, unit, final
                    self.register_row = register_row
                    self.n = len(unit.schunks)
                    self.xps = [
                        xp_pool.tile([1, NT], dt.float32, tag="xps",
                                     name=f"xps{b}{unit.key}{db}")
                        for db in range(4)
                    ]
                    self.ch = 0

                def emit_chunk(self):
                    sc = self.u.schunks[self.ch]
                    slab = encn_tiles[(self.b, sc // 2)]
                    for db in range(4):
                        nc.tensor.matmul(
                            self.xps[db][:],
                            self.u.wct[:, self.ch : self.ch + 1],
                            slab[:, sc % 2, db * NT : (db + 1) * NT],
                            start=(self.ch == 0),
                            stop=(self.ch == self.n - 1),
                        )
                    self.ch += 1
                    if self.ch == self.n:
                        self._drain()

                def _drain(self):
                    u = self.u
                    if not self.final:
                        u.row = rpool.tile([1, D], dt.float32, tag="ctx1",
                                           name=f"row{self.b}{u.key}")
                        for db in range(4):
                            nc.vector.tensor_copy(
                                u.row[0:1, db * NT : (db + 1) * NT],
                                self.xps[db][:],
                            )
                        if self.register_row:
                            row_units[self.b] = u
                        return
                    # final unit (offset 0) + merged row (offset ur.m):
                    # out = (row + e2*ctx_fin) / (Z1 + e2*Zfin), e2 = exp(-m01)
                    ur = row_units[self.b]
                    b = self.b
                    e2 = spool.tile([1, 1], dt.float32, tag="sc", name=f"fe2{b}")
                    nc.scalar.activation(
                        e2[:], ur.m[:], AF.Exp, scale=-1.0,
                        bias=u.m[:] if u.m is not None else pos4_sb[:],
                    )
                    zf2 = spool.tile([1, 1], dt.float32, tag="sc", name=f"fz2{b}")
                    nc.vector.tensor_scalar_mul(zf2[:], u.Z[:], e2[:])
                    z = spool.tile([1, 1], dt.float32, tag="sc", name=f"fz{b}")
                    nc.vector.tensor_scalar_add(z[:], ur.Z[:], zf2[:])
                    rz = spool.tile([1, 1], dt.float32, tag="sc", name=f"fr{b}")
                    nc.vector.reciprocal(rz[:], z[:])
                    s2 = spool.tile([1, 1], dt.float32, tag="sc", name=f"fs2{b}")
                    nc.vector.tensor_scalar_mul(s2[:], e2[:], rz[:])
                    nc.vector.tensor_scalar_mul(ur.row[:], ur.row[:], rz[:])
                    for db in range(4):
                        nc.vector.scalar_tensor_tensor(
                            ur.row[0:1, db * NT : (db + 1) * NT],
                            self.xps[db][:], s2[:],
                            ur.row[0:1, db * NT : (db + 1) * NT],
                            op0=ALU.mult, op1=ALU.add,
                        )
                    nc.scalar.dma_start(out.ap()[b : b + 1, :], ur.row[:])

            def partial_merge(b, u2):
                """Online-merge row-unit u2 into row_units[b] (no normalize)."""
                u1 = row_units[b]
                m = spool.tile([1, 1], dt.float32, tag="sc", name=f"pm{b}")
                nc.vector.tensor_scalar_max(m[:], u1.m[:], u2.m[:])
                nm = spool.tile([1, 1], dt.float32, tag="sc", name=f"pn{b}")
                nc.scalar.mul(nm[:], m[:], -1.0)
                f1 = spool.tile([1, 1], dt.float32, tag="sc", name=f"pf1{b}")
                nc.scalar.activation(f1[:], u1.m[:], AF.Exp, bias=nm[:])
                f2 = spool.tile([1, 1], dt.float32, tag="sc", name=f"pf2{b}")
                nc.scalar.activation(f2[:], u2.m[:], AF.Exp, bias=nm[:])
                zf2 = spool.tile([1, 1], dt.float32, tag="sc", name=f"pz2{b}")
                nc.vector.tensor_scalar_mul(zf2[:], u2.Z[:], f2[:])
                z = spool.tile([1, 1], dt.float32, tag="sc", name=f"pz{b}")
                nc.vector.scalar_tensor_tensor(
                    z[:], u1.Z[:], f1[:], zf2[:], op0=ALU.mult, op1=ALU.add
                )
                nc.vector.tensor_scalar_mul(u1.row[:], u1.row[:], f1[:])
                nc.vector.scalar_tensor_tensor(
                    u1.row[:], u2.row[:], f2[:], u1.row[:],
                    op0=ALU.mult, op1=ALU.add,
                )
                u1.m, u1.Z = m, z

            # ---- main schedule ---------------------------------------
            fetch_enct(0, 0, startup=True)
            fetch_enct(0, 1)
            fetch_encn(0, 0)
            fetch_encn(0, 1)

            def emit_unit_scores(b, key, h, tbs, accs_by_tb,
                                 skip_max=False):
                """Partition-sum the v-weighted accumulators of one unit,
                softmax straight out of PSUM, transpose exp-weights."""
                full = len(tbs) == 2
                u = Unit()
                u.key = key
                wct = wcpool.tile(
                    [128, 4 * len(tbs)], dt.bfloat16,
                    tag="wc" if full else "wcq", name=f"wc{b}{key}",
                )
                wrow = rpool.tile(
                    [1, NT * len(tbs)], dt.bfloat16,
                    tag="wrow" if full else "wrowq", name=f"wrow{b}{key}",
                )
                sps, ms = [], []
                for tb in tbs:
                    sp = xp_pool.tile([1, NT], dt.float32, tag="xps",
                                      name=f"sps{b}{key}{tb}")
                    nc.tensor.matmul(
                        sp[:], ones_sb[:, 0:1], accs_by_tb[tb][:],
                        start=True, stop=True,
                    )
                    if not skip_max:
                        m = spool.tile([1, 1], dt.float32, tag="sc",
                                       name=f"m{b}{key}{tb}")
                        nc.vector.reduce_max(m[:], sp[:], axis=AX.X)
                        ms.append(m)
                    sps.append(sp)
                if skip_max:
                    mh = None       # implicit offset 0: scores are O(1) here
                elif full:
                    mh = spool.tile([1, 1], dt.float32, tag="sc",
                                    name=f"mh{b}{key}")
                    nc.vector.tensor_scalar_max(mh[:], ms[0][:], ms[1][:])
                else:
                    mh = ms[0]
                if mh is not None:
                    nmh = spool.tile([1, 1], dt.float32, tag="sc",
                                     name=f"nmh{b}{key}")
                    nc.scalar.mul(nmh[:], mh[:], -1.0)
                zs = []
                for i, tb in enumerate(tbs):
                    z = spool.tile([1, 1], dt.float32, tag="sc",
                                   name=f"z{b}{key}{tb}")
                    nc.scalar.activation(
                        wrow[0:1, i * NT : (i + 1) * NT], sps[i][:], AF.Exp,
                        bias=neg4_sb[:] if mh is None else nmh[:], accum_out=z[:],
                    )
                    zs.append(z)
                    for ch4 in range(4):
                        col = i * 4 + ch4
                        nc.scalar.dma_start(
                            wct[:, col : col + 1],
                            wrow[0:1, col * 128 : (col + 1) * 128],
                        )
                if full:
                    zh = spool.tile([1, 1], dt.float32, tag="sc",
                                    name=f"zh{b}{key}")
                    nc.vector.tensor_scalar_add(zh[:], zs[0][:], zs[1][:])
                else:
                    zh = zs[0]
                u.m, u.Z, u.wct = mh, zh, wct
                u.schunks = [h * 8 + tb * 4 + c4 for tb in tbs for c4 in range(4)]
                return u

            def emit_energy_chain(b, h, jj, tb, enct_t, acc):
                eps1 = pe_pool.tile([128, NT], dt.float32, tag="eps",
                                    name=f"eps{b}{h}{jj}{tb}")
                for pair in range(PAIRS):
                    nc.tensor.matmul(
                        eps1[:],
                        w8_sb[:, jj, pair, :, :],
                        enct_t[:, pair, :, tb * NT : (tb + 1) * NT],
                        start=(pair == 0),
                        stop=(pair == PAIRS - 1),
                        perf_mode=DR,
                    )
                e_sb = epool.tile([128, NT], dt.bfloat16, tag="e")
                nc.scalar.activation(
                    e_sb[:], eps1[:], AF.Tanh,
                    bias=c_sb[:, jj : jj + 1], scale=ISCALE,
                )
                if jj == 0:
                    nc.vector.tensor_scalar_mul(
                        acc[:], e_sb[:], v_sb[:, jj : jj + 1]
                    )
                else:
                    nc.vector.scalar_tensor_tensor(
                        acc[:], e_sb[:], v_sb[:, jj : jj + 1],
                        acc[:], op0=ALU.mult, op1=ALU.add,
                    )

            pending = None      # context job being interleaved
            prev_half = None    # (b, h, accs) awaiting scores emission

            for idx, (b, h) in enumerate(halves[:3]):
                if idx + 2 < len(halves):
                    fetch_enct(*halves[idx + 2])
                    fetch_encn(*halves[idx + 2])

                enct_t = enct_tiles[(b, h)]
                accs = {
                    tb: apool.tile([128, NT], dt.bfloat16, tag="acc",
                                   name=f"acc{b}{h}{tb}")
                    for tb in range(2)
                }

                for jj in range(JT):
                    for tb in range(2):
                        emit_energy_chain(b, h, jj, tb, enct_t, accs[tb])
                    if jj == 0 and prev_half is not None:
                        assert pending is None or pending.ch == pending.n
                        pb, ph, pa = prev_half
                        pu = emit_unit_scores(pb, f"h{ph}", ph, [0, 1], pa)
                        pending = CtxJob(pb, pu, final=(ph == 1))
                        prev_half = None
                    if pending is not None and pending.ch < pending.n and jj >= 2:
                        pending.emit_chunk()
                        if jj >= 6 and pending.ch < pending.n:
                            pending.emit_chunk()

                prev_half = (b, h, accs)

            # ---- last half (b=1, h=1): tb-major with 512-wide tail unit
            enct_t = enct_tiles[(1, 1)]
            accs3 = {
                tb: apool.tile([128, NT], dt.bfloat16, tag="acc",
                               name=f"acc11{tb}")
                for tb in range(2)
            }
            unit_a_job = None
            for tb in range(2):
                for jj in range(JT):
                    k = tb * JT + jj
                    emit_energy_chain(1, 1, jj, tb, enct_t, accs3[tb])
                    if k == 0:
                        assert pending is None or pending.ch == pending.n
                        pb, ph, pa = prev_half
                        pu = emit_unit_scores(pb, f"h{ph}", ph, [0, 1], pa)
                        pending = CtxJob(pb, pu, final=False)
                    if pending.ch < pending.n and k >= 2:
                        pending.emit_chunk()
                        if k >= 6 and pending.ch < pending.n:
                            pending.emit_chunk()
                    if k == JT:
                        unit_a = emit_unit_scores(1, "q0", 1, [0], accs3)
                        unit_a_job = CtxJob(1, unit_a, final=False,
                                            register_row=False)
                    if unit_a_job is not None and unit_a_job.ch < 4 and k >= 11:
                        unit_a_job.emit_chunk()
                    if k == 15:
                        partial_merge(1, unit_a)

            # tail: the final 512-wide unit
            unit_b = emit_unit_scores(1, "q1", 1, [1], accs3, skip_max=True)
            job_b = CtxJob(1, unit_b, final=True)
            while job_b.ch < job_b.n:
                job_b.emit_chunk()

    nc.compile()
    return nc


def _get_nc():
    if "nc" not in _cache:
        import time

        t0 = time.time()
        _cache["nc"] = _build()
        if os.environ.get("KERNEL_TRACE"):
            print(f"[kernel] bass build+compile: {time.time() - t0:.1f} s")
    return _cache["nc"]


def kernel(hidden, encoder_outputs, attn_w, attn_b, v_w):
    from concourse.bass_utils import run_bass_kernel_spmd

    nc = _get_nc()

    hidden = np.asarray(hidden, dtype=np.float32)
    enc = np.asarray(encoder_outputs, dtype=np.float32)
    attn_w = np.asarray(attn_w, dtype=np.float32)
    attn_b = np.asarray(attn_b, dtype=np.float32)
    v_w = np.asarray(v_w, dtype=np.float32)

    W_h = attn_w[:, :D]
    W_e = attn_w[:, D:]
    c = (hidden @ W_h.T + attn_b).reshape(H)          # [H] fp32, host-side

    v_bf = v_w.reshape(H).astype(BF16)
    Wq = _ef_quant_w(W_e, v_bf.astype(np.float64), SW)  # [H, D] fp8
    # w8[p, pair, i, j] = Wq[j, pair*256 + i*128 + p]
    # w8[jj, p, pair, i, j] = Wq[jj*128+j, pair*256+i*128+p]
    w8 = np.ascontiguousarray(
        Wq.reshape(JT, 128, PAIRS, 2, 128).transpose(0, 4, 2, 3, 1)
    )
    c_cols = np.ascontiguousarray(c.reshape(JT, 128).T)
    v_cols = np.ascontiguousarray(v_bf.astype(np.float32).reshape(JT, 128).T)

    in_maps = []
    for cid in range(N_CORES):
        sl = enc[cid * BPC : (cid + 1) * BPC]           # [BPC, S, D]
        enc8 = (sl * SE).astype(F8)
        encT8 = np.ascontiguousarray(
            enc8.reshape(BPC, 2, HALF, PAIRS, 2, 128).transpose(0, 1, 5, 3, 4, 2)
        )
        encN2 = np.ascontiguousarray(
            sl.astype(BF16).reshape(BPC, 8, 2, 128, D).transpose(0, 1, 3, 2, 4)
        )
        in_maps.append(
            {
                "encT8": encT8,
                "encN2": encN2,
                "w8": w8,
                "c_cols": c_cols,
                "v_cols": v_cols,
            }
        )

    trace = bool(os.environ.get("KERNEL_TRACE"))
    if trace:
        _install_prof_shim()
    res = run_bass_kernel_spmd(
        nc, in_maps, core_ids=list(range(N_CORES)), trace=trace
    )
    if trace:
        _cache["last_exec_time_ns"] = res.exec_time_ns
        print(f"HW exec time: {res.exec_time_ns} ns")

    ctx = np.concatenate([res.results[c]["out"] for c in range(N_CORES)], axis=0)
    return ctx.reshape(B, 1, D).astype(np.float32)


def _install_prof_shim():
    """antenv.axon_hooks is absent from this image; inject it so
    run_bass_kernel_spmd(trace=True) can capture NTFF profiles."""
    import sys
    import types

    if "antenv.axon_hooks" in sys.modules:
        return
    import antenv

    mod = types.ModuleType("antenv.axon_hooks")
    mod._hook = None
    mod.set_axon_ntff_profile_hook = lambda h: setattr(mod, "_hook", h)
    mod.get_axon_ntff_profile_hook = lambda: mod._hook
    sys.modules["antenv.axon_hooks"] = mod
    antenv.axon_hooks = mod
    try:
        from trn_agent_boot.trn_boot import _ntff_profile_via_ctypes

        mod.set_axon_ntff_profile_hook(
            _ntff_profile_via_ctypes("/opt/axon/libaxon_pjrt.so")
        )
    except Exception:
        pass
